# revision 20
# baseline (speedup 1.0000x reference)
"""Self-contained TRN2 Bass kernel for nn_CAM_Module (channel attention).

kernel(x, gamma): x [16,512,64,64] f32, gamma [1] f32 -> [16,512,64,64] f32.
Data-parallel over batch: 2 samples per NeuronCore across 8 cores.

Math: q = x.reshape(B,C,HW); E = q@q.T; softmax(rowmax(E)-E) == softmax(-E)
(shift invariance), computed as exp(rowmin(E)-E)/rowsum; out = gamma*(A@q)+x.

v3 strategy (per core, 2 samples):
  - fp16 I/O, host-prepared: x fp16 [S*C, N] AND its transpose xt fp16
    [S*N, C] (host transpose replaces 256 on-device PE transposes + their
    PSUM bounce evacuations; DMA has the slack, PE does not).
  - E-phase fp16 Gram straight from xt tiles [128, 2, 512] (256
    contraction rows per tile), fp32 PSUM, upper-triangle blocks only
    (E symmetric), mirrored via fp32 PE transposes.
  - softmax via ACT exp(scale=-1, bias=rowmin) -> fp16 weights + fused
    rowsum; fp16 PE transposes of the weights; PSUM->SBUF evacuation
    casts to fp8 in DoubleRow lhsT layout. (fp8 E would be 8e-2 rel err
    - measured offline - so E stays fp16; fp8 A-path is 1.1e-2 << 2e-2.)
  - A-matmul fp8 perf_mode=DoubleRow (~1.5x PE at free=512): rhs q8
    [128,2,N] channel-block pairs cast from the fp16 x tiles.
  - epilogue scalar_tensor_tensor out_fp16 = psum*(gamma/Z) + fp16(x),
    alternating DVE / GpSimd (DVE alone throttled the A-phase pipeline),
    staged into 0.5MB fp16 DMAs.
"""
import sys
if '/opt/trn_rl_repo' not in sys.path:
    sys.path.insert(0, '/opt/trn_rl_repo')
import numpy as np
import concourse.bass as bass
import concourse.tile as tile
import concourse.mybir as mybir
from concourse.masks import make_identity

F32 = mybir.dt.float32
F16 = mybir.dt.float16
F8 = mybir.dt.float8e4

C = 512          # channels
N = 4096         # spatial (64*64)
CB = C // 128    # 4 c-blocks
NT = N // 256    # 16 xt tiles per sample (256 contraction rows each)
NO = N // 512    # 8 output column chunks
NP = 4           # load pieces per row-block (1024 cols each)
PW = N // NP     # piece width
S = 2            # samples per core
DR = mybir.MatmulPerfMode.DoubleRow


def build(nc: bass.Bass):
    x_ext = nc.declare_dram_parameter("x", [S * C, N], F16, isOutput=False)
    # xt rows pair two adjacent xT rows (same host buffer reshaped):
    # row r = [xT[2r], xT[2r+1]], so one 2KB-line DMA fills a [128, 2*C]
    # tile whose halves are valid contraction row-permutations
    xt_ext = nc.declare_dram_parameter("xt", [S * N // 2, 2 * C], F16,
                                       isOutput=False)
    g_ext = nc.declare_dram_parameter("gamma", [1, 1], F32, isOutput=False)
    out_ext = nc.declare_dram_parameter("out", [S * C, N], F16, isOutput=True)
    x_ap = x_ext.ap()
    xt_ap = xt_ext.ap()
    out_ap = out_ext.ap()
    EW = [512 - 128 * m for m in range(CB)]

    with tile.TileContext(nc) as tc:
        with (
            tc.tile_pool(name="const", bufs=1) as const,
            tc.tile_pool(name="q16", bufs=2 * CB * NP) as q16p,
            tc.tile_pool(name="xt", bufs=NT + 8) as xtp,
            tc.tile_pool(name="q8", bufs=2 * 2) as q8p_pool,
            tc.tile_pool(name="esb", bufs=2) as esbp,
            tc.tile_pool(name="expn", bufs=2) as expnp,
            tc.tile_pool(name="expt", bufs=2 * 2) as exptp,
            tc.tile_pool(name="vecs", bufs=4 * CB) as vecs,
            tc.tile_pool(name="outs", bufs=7) as outsp,
            tc.tile_pool(name="ps_bounce", bufs=1, space="PSUM") as ps_t,
            tc.tile_pool(name="ps_e", bufs=1, space="PSUM") as ps_e,
            tc.tile_pool(name="ps_o", bufs=3, space="PSUM") as ps_o,
        ):
            ident = const.tile([128, 128], F16)
            make_identity(nc, ident)
            ident32 = const.tile([128, 128], F32)
            make_identity(nc, ident32)
            gbc = const.tile([128, 1], F32)
            nc.gpsimd.dma_start(out=gbc, in_=g_ext.ap().to_broadcast((128, 1)))

            st = [dict() for _ in range(S)]

            def load(s):
                # x pieces for the fp8 cast + the epilogue residual
                q16 = [[None] * NP for _ in range(CB)]
                for p in range(NP):
                    for cb in range(CB):
                        qc = q16p.tile([128, PW], F16, tag="q16",
                                       name=f"q16_{s}_{cb}_{p}")
                        # gpsimd queue: keeps x-load dispatch off the sync
                        # queue so xt loads (E-phase critical) issue first
                        nc.gpsimd.dma_start(
                            out=qc,
                            in_=x_ap[
                                s * C + cb * 128 : s * C + (cb + 1) * 128,
                                p * PW : (p + 1) * PW,
                            ],
                        )
                        q16[cb][p] = qc
                st[s]["q16"] = q16

            def loadT(s):
                # xt tile k: [p, o*C:(o+1)*C] = xT[2*(k*128+p) + o] - any
                # consistent row permutation is fine for the Gram
                xts = []
                for k in range(NT):
                    xt_t = xtp.tile([128, 2 * C], F16, tag="xt",
                                    name=f"xt_{s}_{k}")
                    base = (s * N) // 2 + k * 128
                    nc.sync.dma_start(
                        out=xt_t, in_=xt_ap[base : base + 128, :],
                    )
                    xts.append(xt_t)
                st[s]["xt"] = xts

            def cast8(s):
                # fp8 copy of q in DoubleRow rhs layout: q8[h][p, o, n] =
                # q[channel 256h+128o+p][n]
                q16 = st[s]["q16"]
                q8 = []
                for h in range(2):
                    q8t = q8p_pool.tile([128, 2, N], F8, tag="q8",
                                        name=f"q8_{s}_{h}")
                    for o in range(2):
                        for p in range(NP):
                            src = q16[2 * h + o][p]
                            dst = q8t[:, o, p * PW : (p + 1) * PW]
                            if (o * NP + p) % 2 == 0:
                                nc.scalar.copy(dst, src[:])
                            else:
                                nc.vector.tensor_copy(dst, src[:])
                    q8.append(q8t)
                st[s]["q8"] = q8

            def emm(s, k):
                # symmetric Gram accumulation: upper-triangle blocks only
                if "E" not in st[s]:
                    st[s]["E"] = ps_e.tile([128, CB, 512], F32, tag="E",
                                           name=f"E_{s}")
                E = st[s]["E"]
                xt_t = st[s]["xt"][k]
                for o in range(2):
                    for m in range(CB):
                        nc.tensor.matmul(
                            E[:, m, 0 : EW[m]],
                            lhsT=xt_t[:, o * C + m * 128 : o * C + (m + 1) * 128],
                            rhs=xt_t[:, o * C + m * 128 : o * C + 512],
                            start=(k == 0 and o == 0),
                            stop=(k == NT - 1 and o == 1),
                        )

            def softmax(s):
                # rebuild full E rows in SBUF (mirror lower triangle),
                # then exp(rowmin - E) + fused rowsum
                E = st[s]["E"]
                E_sb = esbp.tile([128, CB, 512], F32, tag="esb",
                                 name=f"esb_{s}")
                for m in range(CB):
                    nc.scalar.copy(E_sb[:, m, m * 128 : 512],
                                   E[:, m, 0 : EW[m]])
                for i in range(CB):
                    for j in range(i):
                        tb = ps_o.tile([128, 128], F32, tag="acc",
                                       name=f"tb_{s}_{i}_{j}")
                        nc.tensor.transpose(
                            tb[:], E_sb[:, j, i * 128 : (i + 1) * 128], ident32
                        )
                        if (i + j) % 2 == 0:
                            nc.scalar.copy(
                                E_sb[:, i, j * 128 : (j + 1) * 128], tb[:])
                        else:
                            nc.vector.tensor_copy(
                                E_sb[:, i, j * 128 : (j + 1) * 128], tb[:])
                expn = expnp.tile([128, CB, 512], F16, tag="expn",
                                  name=f"expn_{s}")
                scales = []
                for m in range(CB):
                    mv = vecs.tile([128, 1], F32, tag="mv", name=f"mv_{s}_{m}")
                    nc.vector.tensor_reduce(
                        mv, E_sb[:, m, :], axis=mybir.AxisListType.X,
                        op=mybir.AluOpType.min,
                    )
                    Z = vecs.tile([128, 1], F32, tag="Z", name=f"Z_{s}_{m}")
                    nc.scalar.activation(
                        expn[:, m, :],
                        E_sb[:, m, :],
                        mybir.ActivationFunctionType.Exp,
                        bias=mv,
                        scale=-1.0,
                        accum_out=Z,
                    )
                    rz = vecs.tile([128, 1], F32, tag="rz", name=f"rz_{s}_{m}")
                    nc.vector.reciprocal(rz, Z)
                    sc = vecs.tile([128, 1], F32, tag="sc", name=f"sc_{s}_{m}")
                    nc.vector.tensor_mul(sc, rz, gbc)  # gamma / Z
                    scales.append(sc)
                st[s]["expn"] = expn
                st[s]["scales"] = scales

            def expTf(s):
                # fp16 transposes of the weights; the PSUM->SBUF evacuation
                # casts to fp8 in DoubleRow lhsT layout:
                # et[h][p, o, cb*128+c] = exp[cb-block][c, 128*(2h+o)+p]
                expn = st[s]["expn"]
                expT = []
                for h in range(2):
                    bounce = ps_t.tile([128, 2, CB, 128], F16, tag="bounce",
                                       name=f"ebounce_{s}_{h}")
                    for o in range(2):
                        j = 2 * h + o
                        for cb in range(CB):
                            nc.tensor.transpose(
                                bounce[:, o, cb, :],
                                expn[:, cb, j * 128 : (j + 1) * 128],
                                ident,
                            )
                    et = exptp.tile([128, 2, CB * 128], F8, tag="expT",
                                    name=f"expT_{s}_{h}")
                    if h % 2 == 0:
                        nc.scalar.copy(et[:], bounce[:, :, :, :])
                    else:
                        nc.vector.tensor_copy(et[:], bounce[:, :, :, :])
                    expT.append(et)
                st[s]["expT"] = expT

            def aphase(s, cbs=range(CB)):
                # out = gamma/Z * (exp @ q) + x via fp8 DoubleRow matmuls;
                # DVE epilogue; staged 0.5MB fp16 DMAs
                q16, q8 = st[s]["q16"], st[s]["q8"]
                expT, scales = st[s]["expT"], st[s]["scales"]
                ostage = {}
                for cb in cbs:
                    for no in range(NO):
                        npc, nof = no // (PW // 512), (no % (PW // 512)) * 512
                        psl = slice(nof, nof + 512)
                        acc = ps_o.tile([128, 512], F32, tag="acc",
                                        name=f"acc_{s}_{no}_{cb}")
                        for h in range(2):
                            nc.tensor.matmul(
                                acc[:],
                                lhsT=expT[h][:, :, cb * 128 : (cb + 1) * 128],
                                rhs=q8[h][:, :, no * 512 : (no + 1) * 512],
                                start=(h == 0),
                                stop=(h == 1),
                                perf_mode=DR,
                            )
                        half = no // (NO // 2)
                        if (cb, half) not in ostage:
                            ot = outsp.tile([128, (NO // 2) * 512], F16,
                                            tag="ot", name=f"ot_{s}_{cb}_{half}")
                            ostage[(cb, half)] = ot
                        ot = ostage[(cb, half)]
                        osl = slice((no % (NO // 2)) * 512,
                                    (no % (NO // 2) + 1) * 512)
                        nc.vector.scalar_tensor_tensor(
                            out=ot[:, osl],
                            in0=acc[:],
                            scalar=scales[cb],
                            in1=q16[cb][npc][:, psl],
                            op0=mybir.AluOpType.mult,
                            op1=mybir.AluOpType.add,
                        )
                        if no % (NO // 2) == NO // 2 - 1:
                            nc.sync.dma_start(
                                out=out_ap[
                                    s * C + cb * 128 : s * C + (cb + 1) * 128,
                                    half * (NO // 2) * 512 :
                                    (half + 1) * (NO // 2) * 512,
                                ],
                                in_=ot[:],
                            )

            # ---- interleaved emission schedule -----------------------
            # xt loads first (E-phase critical path); x loads go out on
            # the gpsimd queue in parallel
            loadT(0)
            loadT(1)
            load(0)
            load(1)
            for k in range(NT):
                emm(0, k)
            softmax(0)
            expTf(0)
            cast8(0)
            cast8(1)
            for k in range(NT):
                emm(1, k)
            # A(s0) emitted around softmax(1) so its matmuls fill the PE
            # while sample-1's softmax chain runs on ACT/DVE
            aphase(0, range(0, 2))
            softmax(1)
            aphase(0, range(2, CB))
            expTf(1)
            aphase(1)
    return nc


def _split_excess_waits(nc, max_waits=1):
    """This container's walrus rejects >1 sync-wait on one instruction
    ("Too many sync wait commands"); hoist extras onto standalone
    InstEventSemaphore preludes on the same engine."""
    n = 0
    for fn in nc.m.functions:
        for bb in fn.blocks:
            out = []
            for inst in bb.instructions:
                si = inst.sync_info
                if si is not None and si.on_wait and len(si.on_wait) > max_waits:
                    waits = list(si.on_wait)
                    head, keep = waits[:-max_waits], waits[-max_waits:]
                    for i, w in enumerate(head):
                        ev = mybir.InstEventSemaphore(
                            name=f"{inst.name}-wsplit{i}", ins=[], outs=[])
                        ev.engine = inst.engine
                        ev.sync_info = mybir.SyncInfo(on_wait=[w], on_update=[])
                        out.append(ev)
                        n += 1
                    inst.sync_info = mybir.SyncInfo(
                        on_wait=keep, on_update=list(si.on_update))
                out.append(inst)
            bb.instructions[:] = out
    return n


_cache = {}


def _get_nc():
    if 'nc' not in _cache:
        nc = bass.Bass()
        build(nc)
        _split_excess_waits(nc)
        _cache['nc'] = nc
    return _cache['nc']


def make_in_maps(x: np.ndarray, gamma: np.ndarray, n_cores: int = 8):
    B, CH, H, W = x.shape          # (16, 512, 64, 64)
    NSP = H * W
    SS = B // n_cores
    g = np.ascontiguousarray(gamma, dtype=np.float32).reshape(1, 1)
    x16 = np.ascontiguousarray(x, dtype=np.float16).reshape(B, CH, NSP)
    xt16 = np.ascontiguousarray(x16.transpose(0, 2, 1))   # [B, N, C]
    return [
        {
            "x": np.ascontiguousarray(
                x16[i * SS : (i + 1) * SS].reshape(SS * CH, NSP)
            ),
            "xt": np.ascontiguousarray(
                xt16[i * SS : (i + 1) * SS].reshape(SS * NSP // 2, 2 * CH)
            ),
            "gamma": g,
        }
        for i in range(n_cores)
    ]


def kernel(x: np.ndarray, gamma: np.ndarray) -> np.ndarray:
    from concourse.bass_utils import run_bass_kernel_spmd

    B, CH, H, W = x.shape          # (16, 512, 64, 64)
    M = 8                          # cores
    SS = B // M                    # samples per core
    nc = _get_nc()
    in_maps = make_in_maps(x, gamma, M)
    res = run_bass_kernel_spmd(nc, in_maps, core_ids=list(range(M)))
    out = np.concatenate(
        [
            res.results[i]["out"].astype(np.float32).reshape(SS, CH, H, W)
            for i in range(M)
        ],
        axis=0,
    )
    return np.ascontiguousarray(out, dtype=np.float32)


# revision 21
# speedup vs baseline: 1.0119x; 1.0119x over previous
"""Self-contained TRN2 Bass kernel for nn_CAM_Module (channel attention).

kernel(x, gamma): x [16,512,64,64] f32, gamma [1] f32 -> [16,512,64,64] f32.
Data-parallel over batch: 2 samples per NeuronCore across 8 cores.

Math: q = x.reshape(B,C,HW); E = q@q.T; softmax(rowmax(E)-E) == softmax(-E)
(shift invariance), computed as exp(rowmin(E)-E)/rowsum; out = gamma*(A@q)+x.

v5 strategy (per core, 2 samples):
  - The device computes ONLY gamma/Z * (exp @ q); the residual +x is added
    on the host in f32 (better precision AND -4MB/core of device traffic).
  - Inputs: xt fp16 [S*N/2, 2C] = host-transposed x, paired rows so one
    2KB-line DMA fills a [128, 1024] tile (row permutations are harmless
    for the Gram); x8 fp8e4 [S*C, N] = host-cast x for the A-phase rhs
    (values <=5.5 so OCP e4m3fn bytes == TRN fp8e4). 12MB in, 8MB out.
  - E-phase fp16 Gram straight from xt tiles, fp32 PSUM, upper-triangle
    blocks only (E symmetric), mirrored via fp32 PE transposes. (fp8 E
    would be 8e-2 rel err - measured offline - so E stays fp16.)
  - softmax via ACT exp(scale=-1, bias=rowmin) -> fp16 weights + fused
    rowsum; fp16 PE transposes of the weights; PSUM->SBUF evacuation
    casts to fp8 in DoubleRow lhsT layout.
  - A-matmul fp8 perf_mode=DoubleRow (~1.5x PE at free=512), rhs x8 in
    [128,2,N] channel-block pairs loaded directly from DRAM.
  - epilogue: out_fp16 = psum * (gamma/Z), alternating ACT scale-copy /
    DVE tensor-scalar; staged 0.5MB fp16 DMAs.
"""
import sys
if '/opt/trn_rl_repo' not in sys.path:
    sys.path.insert(0, '/opt/trn_rl_repo')
import numpy as np
import concourse.bass as bass
import concourse.tile as tile
import concourse.mybir as mybir
from concourse.masks import make_identity

F32 = mybir.dt.float32
F16 = mybir.dt.float16
F8 = mybir.dt.float8e4

C = 512          # channels
N = 4096         # spatial (64*64)
CB = C // 128    # 4 c-blocks
NT = N // 256    # 16 xt tiles per sample (256 contraction rows each)
NO = N // 512    # 8 output column chunks
S = 2            # samples per core
DR = mybir.MatmulPerfMode.DoubleRow


def build(nc: bass.Bass):
    # xt rows pair two adjacent xT rows (same host buffer reshaped):
    # row r = [xT[2r], xT[2r+1]]
    xt_ext = nc.declare_dram_parameter("xt", [S * N // 2, 2 * C], F16,
                                       isOutput=False)
    x8_ext = nc.declare_dram_parameter("x8", [S * C, N], F8, isOutput=False)
    g_ext = nc.declare_dram_parameter("gamma", [1, 1], F32, isOutput=False)
    out_ext = nc.declare_dram_parameter("out", [S * C, N], F16, isOutput=True)
    xt_ap = xt_ext.ap()
    x8_ap = x8_ext.ap()
    out_ap = out_ext.ap()
    EW = [512 - 128 * m for m in range(CB)]

    with tile.TileContext(nc) as tc:
        with (
            tc.tile_pool(name="const", bufs=1) as const,
            tc.tile_pool(name="xt", bufs=NT + 8) as xtp,
            tc.tile_pool(name="q8", bufs=2 * 2) as q8p_pool,
            tc.tile_pool(name="esb", bufs=2) as esbp,
            tc.tile_pool(name="expn", bufs=2) as expnp,
            tc.tile_pool(name="expt", bufs=2 * 2) as exptp,
            tc.tile_pool(name="vecs", bufs=4 * CB) as vecs,
            tc.tile_pool(name="outs", bufs=8) as outsp,
            tc.tile_pool(name="ps_bounce", bufs=1, space="PSUM") as ps_t,
            tc.tile_pool(name="ps_e", bufs=1, space="PSUM") as ps_e,
            tc.tile_pool(name="ps_o", bufs=3, space="PSUM") as ps_o,
        ):
            ident = const.tile([128, 128], F16)
            make_identity(nc, ident)
            ident32 = const.tile([128, 128], F32)
            make_identity(nc, ident32)
            gbc = const.tile([128, 1], F32)
            nc.gpsimd.dma_start(out=gbc, in_=g_ext.ap().to_broadcast((128, 1)))

            st = [dict() for _ in range(S)]

            def loadT(s):
                # xt tile k: [p, o*C:(o+1)*C] = xT[2*(k*128+p) + o] - any
                # consistent row permutation is fine for the Gram
                xts = []
                for k in range(NT):
                    xt_t = xtp.tile([128, 2 * C], F16, tag="xt",
                                    name=f"xt_{s}_{k}")
                    base = (s * N) // 2 + k * 128
                    nc.sync.dma_start(
                        out=xt_t, in_=xt_ap[base : base + 128, :],
                    )
                    xts.append(xt_t)
                st[s]["xt"] = xts

            def load8(s):
                # fp8 q in DoubleRow rhs layout: q8[h][p, o, n] =
                # x8[channel 256h+128o+p][n]; 4KB-line DMAs, gpsimd queue
                # to keep dispatch off the xt (sync) queue
                q8 = []
                for h in range(2):
                    q8t = q8p_pool.tile([128, 2, N], F8, tag="q8",
                                        name=f"q8_{s}_{h}")
                    for o in range(2):
                        r0 = s * C + 256 * h + 128 * o
                        nc.gpsimd.dma_start(
                            out=q8t[:, o, :],
                            in_=x8_ap[r0 : r0 + 128, :],
                        )
                    q8.append(q8t)
                st[s]["q8"] = q8

            def emm(s, k):
                # symmetric Gram accumulation: upper-triangle blocks only
                if "E" not in st[s]:
                    st[s]["E"] = ps_e.tile([128, CB, 512], F32, tag="E",
                                           name=f"E_{s}")
                E = st[s]["E"]
                xt_t = st[s]["xt"][k]
                for o in range(2):
                    for m in range(CB):
                        nc.tensor.matmul(
                            E[:, m, 0 : EW[m]],
                            lhsT=xt_t[:, o * C + m * 128 : o * C + (m + 1) * 128],
                            rhs=xt_t[:, o * C + m * 128 : o * C + 512],
                            start=(k == 0 and o == 0),
                            stop=(k == NT - 1 and o == 1),
                        )

            def softmax(s):
                # rebuild full E rows in SBUF (mirror lower triangle),
                # then exp(rowmin - E) + fused rowsum
                E = st[s]["E"]
                E_sb = esbp.tile([128, CB, 512], F32, tag="esb",
                                 name=f"esb_{s}")
                for m in range(CB):
                    nc.scalar.copy(E_sb[:, m, m * 128 : 512],
                                   E[:, m, 0 : EW[m]])
                for i in range(CB):
                    for j in range(i):
                        tb = ps_o.tile([128, 128], F32, tag="acc",
                                       name=f"tb_{s}_{i}_{j}")
                        nc.tensor.transpose(
                            tb[:], E_sb[:, j, i * 128 : (i + 1) * 128], ident32
                        )
                        if (i + j) % 2 == 0:
                            nc.scalar.copy(
                                E_sb[:, i, j * 128 : (j + 1) * 128], tb[:])
                        else:
                            nc.vector.tensor_copy(
                                E_sb[:, i, j * 128 : (j + 1) * 128], tb[:])
                expn = expnp.tile([128, CB, 512], F16, tag="expn",
                                  name=f"expn_{s}")
                scales = []
                for m in range(CB):
                    mv = vecs.tile([128, 1], F32, tag="mv", name=f"mv_{s}_{m}")
                    nc.vector.tensor_reduce(
                        mv, E_sb[:, m, :], axis=mybir.AxisListType.X,
                        op=mybir.AluOpType.min,
                    )
                    Z = vecs.tile([128, 1], F32, tag="Z", name=f"Z_{s}_{m}")
                    nc.scalar.activation(
                        expn[:, m, :],
                        E_sb[:, m, :],
                        mybir.ActivationFunctionType.Exp,
                        bias=mv,
                        scale=-1.0,
                        accum_out=Z,
                    )
                    rz = vecs.tile([128, 1], F32, tag="rz", name=f"rz_{s}_{m}")
                    nc.vector.reciprocal(rz, Z)
                    sc = vecs.tile([128, 1], F32, tag="sc", name=f"sc_{s}_{m}")
                    nc.vector.tensor_mul(sc, rz, gbc)  # gamma / Z
                    scales.append(sc)
                st[s]["expn"] = expn
                st[s]["scales"] = scales

            def expTf(s):
                # fp16 transposes of the weights; the PSUM->SBUF evacuation
                # casts to fp8 in DoubleRow lhsT layout:
                # et[h][p, o, cb*128+c] = exp[cb-block][c, 128*(2h+o)+p]
                expn = st[s]["expn"]
                expT = []
                for h in range(2):
                    bounce = ps_t.tile([128, 2, CB, 128], F16, tag="bounce",
                                       name=f"ebounce_{s}_{h}")
                    for o in range(2):
                        j = 2 * h + o
                        for cb in range(CB):
                            nc.tensor.transpose(
                                bounce[:, o, cb, :],
                                expn[:, cb, j * 128 : (j + 1) * 128],
                                ident,
                            )
                    et = exptp.tile([128, 2, CB * 128], F8, tag="expT",
                                    name=f"expT_{s}_{h}")
                    if h % 2 == 0:
                        nc.scalar.copy(et[:], bounce[:, :, :, :])
                    else:
                        nc.vector.tensor_copy(et[:], bounce[:, :, :, :])
                    expT.append(et)
                st[s]["expT"] = expT

            def aphase(s, cbs=range(CB)):
                # psum = exp @ q via fp8 DoubleRow matmuls; epilogue
                # out_fp16 = psum * (gamma/Z) alternating ACT / DVE;
                # staged 0.5MB fp16 DMAs (host adds the +x residual)
                q8 = st[s]["q8"]
                expT, scales = st[s]["expT"], st[s]["scales"]
                ostage = {}
                for cb in cbs:
                    for no in range(NO):
                        acc = ps_o.tile([128, 512], F32, tag="acc",
                                        name=f"acc_{s}_{no}_{cb}")
                        for h in range(2):
                            nc.tensor.matmul(
                                acc[:],
                                lhsT=expT[h][:, :, cb * 128 : (cb + 1) * 128],
                                rhs=q8[h][:, :, no * 512 : (no + 1) * 512],
                                start=(h == 0),
                                stop=(h == 1),
                                perf_mode=DR,
                            )
                        half = no // (NO // 2)
                        if (cb, half) not in ostage:
                            ot = outsp.tile([128, (NO // 2) * 512], F16,
                                            tag="ot", name=f"ot_{s}_{cb}_{half}")
                            ostage[(cb, half)] = ot
                        ot = ostage[(cb, half)]
                        osl = slice((no % (NO // 2)) * 512,
                                    (no % (NO // 2) + 1) * 512)
                        if no % 2 == 0:
                            nc.scalar.activation(
                                ot[:, osl], acc[:],
                                mybir.ActivationFunctionType.Copy,
                                scale=scales[cb],
                            )
                        else:
                            nc.vector.tensor_scalar_mul(
                                ot[:, osl], acc[:], scales[cb])
                        if no % (NO // 2) == NO // 2 - 1:
                            nc.sync.dma_start(
                                out=out_ap[
                                    s * C + cb * 128 : s * C + (cb + 1) * 128,
                                    half * (NO // 2) * 512 :
                                    (half + 1) * (NO // 2) * 512,
                                ],
                                in_=ot[:],
                            )

            # ---- interleaved emission schedule -----------------------
            loadT(0)
            load8(0)
            loadT(1)
            load8(1)
            for k in range(NT):
                emm(0, k)
            softmax(0)
            expTf(0)
            for k in range(NT):
                emm(1, k)
            # A(s0) emitted around softmax(1) so its matmuls fill the PE
            # while sample-1's softmax chain runs on ACT/DVE
            aphase(0, range(0, 2))
            softmax(1)
            aphase(0, range(2, CB))
            expTf(1)
            aphase(1)
    return nc


def _split_excess_waits(nc, max_waits=1):
    """This container's walrus rejects >1 sync-wait on one instruction
    ("Too many sync wait commands"); hoist extras onto standalone
    InstEventSemaphore preludes on the same engine."""
    n = 0
    for fn in nc.m.functions:
        for bb in fn.blocks:
            out = []
            for inst in bb.instructions:
                si = inst.sync_info
                if si is not None and si.on_wait and len(si.on_wait) > max_waits:
                    waits = list(si.on_wait)
                    head, keep = waits[:-max_waits], waits[-max_waits:]
                    for i, w in enumerate(head):
                        ev = mybir.InstEventSemaphore(
                            name=f"{inst.name}-wsplit{i}", ins=[], outs=[])
                        ev.engine = inst.engine
                        ev.sync_info = mybir.SyncInfo(on_wait=[w], on_update=[])
                        out.append(ev)
                        n += 1
                    inst.sync_info = mybir.SyncInfo(
                        on_wait=keep, on_update=list(si.on_update))
                out.append(inst)
            bb.instructions[:] = out
    return n


_cache = {}


def _get_nc():
    if 'nc' not in _cache:
        nc = bass.Bass()
        build(nc)
        _split_excess_waits(nc)
        _cache['nc'] = nc
    return _cache['nc']


def make_in_maps(x: np.ndarray, gamma: np.ndarray, n_cores: int = 8):
    import ml_dtypes

    B, CH, H, W = x.shape          # (16, 512, 64, 64)
    NSP = H * W
    SS = B // n_cores
    g = np.ascontiguousarray(gamma, dtype=np.float32).reshape(1, 1)
    x16 = np.ascontiguousarray(x, dtype=np.float16).reshape(B, CH, NSP)
    # A-phase rhs: e4m3 (|x|<=5.5 so OCP e4m3fn bytes == TRN fp8e4)
    x8 = x16.astype(ml_dtypes.float8_e4m3fn)
    xt16 = np.ascontiguousarray(x16.transpose(0, 2, 1))   # [B, N, C]
    return [
        {
            "xt": np.ascontiguousarray(
                xt16[i * SS : (i + 1) * SS].reshape(SS * NSP // 2, 2 * CH)
            ),
            "x8": np.ascontiguousarray(
                x8[i * SS : (i + 1) * SS].reshape(SS * CH, NSP)
            ),
            "gamma": g,
        }
        for i in range(n_cores)
    ]


def kernel(x: np.ndarray, gamma: np.ndarray) -> np.ndarray:
    from concourse.bass_utils import run_bass_kernel_spmd

    B, CH, H, W = x.shape          # (16, 512, 64, 64)
    M = 8                          # cores
    SS = B // M                    # samples per core
    nc = _get_nc()
    in_maps = make_in_maps(x, gamma, M)
    res = run_bass_kernel_spmd(nc, in_maps, core_ids=list(range(M)))
    out = np.concatenate(
        [
            res.results[i]["out"].astype(np.float32).reshape(SS, CH, H, W)
            for i in range(M)
        ],
        axis=0,
    )
    # residual added host-side in f32
    return np.ascontiguousarray(out + x.astype(np.float32), dtype=np.float32)


# revision 25
# speedup vs baseline: 1.2039x; 1.1897x over previous
"""Self-contained TRN2 Bass kernel for nn_CAM_Module (channel attention).

kernel(x, gamma): x [16,512,64,64] f32, gamma [1] f32 -> [16,512,64,64] f32.
Data-parallel over batch: 2 samples per NeuronCore across 8 cores.

Math: q = x.reshape(B,C,HW); E = q@q.T; softmax(rowmax(E)-E) == softmax(-E)
(shift invariance), computed as exp(rowmin(E)-E)/rowsum; out = gamma*(A@q)+x.

v5 strategy (per core, 2 samples):
  - The device computes ONLY gamma/Z * (exp @ q); the residual +x is added
    on the host in f32 (better precision AND -4MB/core of device traffic).
  - Inputs: xt fp16 [S*N/2, 2C] = host-transposed x, paired rows so one
    2KB-line DMA fills a [128, 1024] tile (row permutations are harmless
    for the Gram); x8 fp8e4 [S*C, N] = host-cast x for the A-phase rhs
    (values <=5.5 so OCP e4m3fn bytes == TRN fp8e4). 12MB in, 8MB out.
  - E-phase fp16 Gram straight from xt tiles, fp32 PSUM, upper-triangle
    blocks only (E symmetric), mirrored via fp32 PE transposes. (fp8 E
    would be 8e-2 rel err - measured offline - so E stays fp16.)
  - softmax via ACT exp(scale=-1, bias=rowmin) -> fp16 weights + fused
    rowsum; fp16 PE transposes of the weights; PSUM->SBUF evacuation
    casts to fp8 in DoubleRow lhsT layout.
  - A-matmul fp8 perf_mode=DoubleRow (~1.5x PE at free=512), rhs x8 in
    [128,2,N] channel-block pairs loaded directly from DRAM.
  - epilogue: out_fp16 = psum * (gamma/Z), alternating ACT scale-copy /
    DVE tensor-scalar; staged 0.5MB fp16 DMAs.
"""
import sys
if '/opt/trn_rl_repo' not in sys.path:
    sys.path.insert(0, '/opt/trn_rl_repo')
import numpy as np
import concourse.bass as bass
import concourse.tile as tile
import concourse.mybir as mybir
from concourse.masks import make_identity

F32 = mybir.dt.float32
F16 = mybir.dt.float16
F8 = mybir.dt.float8e4

C = 512          # channels
N = 4096         # spatial (64*64)
CB = C // 128    # 4 c-blocks
NT = N // 256    # 16 xt tiles per sample (256 contraction rows each)
NO = N // 512    # 8 output column chunks
S = 2            # samples per core
DR = mybir.MatmulPerfMode.DoubleRow


def build(nc: bass.Bass):
    # xt rows pair two adjacent xT rows (same host buffer reshaped):
    # row r = [xT[2r], xT[2r+1]]
    xt_ext = nc.declare_dram_parameter("xt", [S * N // 2, 2 * C], F16,
                                       isOutput=False)
    x8_ext = nc.declare_dram_parameter("x8", [S * C, N], F8, isOutput=False)
    g_ext = nc.declare_dram_parameter("gamma", [1, 1], F32, isOutput=False)
    out_ext = nc.declare_dram_parameter("out", [S * C, N], F16, isOutput=True)
    xt_ap = xt_ext.ap()
    x8_ap = x8_ext.ap()
    out_ap = out_ext.ap()
    EW = [512 - 128 * m for m in range(CB)]

    with tile.TileContext(nc) as tc:
        with (
            tc.tile_pool(name="const", bufs=1) as const,
            tc.tile_pool(name="xt", bufs=NT + 8) as xtp,
            tc.tile_pool(name="q8", bufs=2 * 2) as q8p_pool,
            tc.tile_pool(name="esb", bufs=2) as esbp,
            tc.tile_pool(name="expn", bufs=2) as expnp,
            tc.tile_pool(name="expt", bufs=2 * 2) as exptp,
            tc.tile_pool(name="vecs", bufs=4 * CB) as vecs,
            tc.tile_pool(name="outs", bufs=8) as outsp,
            tc.tile_pool(name="ps_e", bufs=1, space="PSUM") as ps_e,
            tc.tile_pool(name="ps_o", bufs=4, space="PSUM") as ps_o,
        ):
            ident = const.tile([128, 128], F16)
            make_identity(nc, ident)
            ident32 = const.tile([128, 128], F32)
            make_identity(nc, ident32)
            gbc = const.tile([128, 1], F32)
            nc.gpsimd.dma_start(out=gbc, in_=g_ext.ap().to_broadcast((128, 1)))

            st = [dict() for _ in range(S)]

            def loadT(s):
                # xt tile k: [p, o*C:(o+1)*C] = xT[2*(k*128+p) + o] - any
                # consistent row permutation is fine for the Gram
                xts = []
                for k in range(NT):
                    xt_t = xtp.tile([128, 2 * C], F16, tag="xt",
                                    name=f"xt_{s}_{k}")
                    base = (s * N) // 2 + k * 128
                    nc.sync.dma_start(
                        out=xt_t, in_=xt_ap[base : base + 128, :],
                    )
                    xts.append(xt_t)
                st[s]["xt"] = xts

            def load8(s):
                # fp8 q in DoubleRow rhs layout: q8[h][p, o, n] =
                # x8[channel 256h+128o+p][n]; 4KB-line DMAs, gpsimd queue
                # to keep dispatch off the xt (sync) queue
                q8 = []
                for h in range(2):
                    q8t = q8p_pool.tile([128, 2, N], F8, tag="q8",
                                        name=f"q8_{s}_{h}")
                    for o in range(2):
                        r0 = s * C + 256 * h + 128 * o
                        nc.gpsimd.dma_start(
                            out=q8t[:, o, :],
                            in_=x8_ap[r0 : r0 + 128, :],
                        )
                    q8.append(q8t)
                st[s]["q8"] = q8

            def emm(s, k):
                # symmetric Gram accumulation: upper-triangle blocks only
                if "E" not in st[s]:
                    st[s]["E"] = ps_e.tile([128, CB, 512], F32, tag="E",
                                           name=f"E_{s}")
                E = st[s]["E"]
                xt_t = st[s]["xt"][k]
                for o in range(2):
                    for m in range(CB):
                        nc.tensor.matmul(
                            E[:, m, 0 : EW[m]],
                            lhsT=xt_t[:, o * C + m * 128 : o * C + (m + 1) * 128],
                            rhs=xt_t[:, o * C + m * 128 : o * C + 512],
                            start=(k == 0 and o == 0),
                            stop=(k == NT - 1 and o == 1),
                        )

            def softmax(s):
                # rebuild full E rows in SBUF (mirror lower triangle),
                # then exp(rowmin - E) + fused rowsum
                E = st[s]["E"]
                E_sb = esbp.tile([128, CB, 512], F32, tag="esb",
                                 name=f"esb_{s}")
                for m in range(CB):
                    if m % 2 == 0:
                        nc.scalar.copy(E_sb[:, m, m * 128 : 512],
                                       E[:, m, 0 : EW[m]])
                    else:
                        nc.vector.tensor_copy(E_sb[:, m, m * 128 : 512],
                                              E[:, m, 0 : EW[m]])
                for i in range(CB):
                    for j in range(i):
                        tb = ps_o.tile([128, 128], F32, tag="acc",
                                       name=f"tb_{s}_{i}_{j}")
                        nc.tensor.transpose(
                            tb[:], E_sb[:, j, i * 128 : (i + 1) * 128], ident32
                        )
                        if (i + j) % 2 == 0:
                            nc.scalar.copy(
                                E_sb[:, i, j * 128 : (j + 1) * 128], tb[:])
                        else:
                            nc.vector.tensor_copy(
                                E_sb[:, i, j * 128 : (j + 1) * 128], tb[:])
                expn = expnp.tile([128, CB, 512], F16, tag="expn",
                                  name=f"expn_{s}")
                scales = []
                for m in range(CB):
                    mv = vecs.tile([128, 1], F32, tag="mv", name=f"mv_{s}_{m}")
                    nc.vector.tensor_reduce(
                        mv, E_sb[:, m, :], axis=mybir.AxisListType.X,
                        op=mybir.AluOpType.min,
                    )
                    Z = vecs.tile([128, 1], F32, tag="Z", name=f"Z_{s}_{m}")
                    nc.scalar.activation(
                        expn[:, m, :],
                        E_sb[:, m, :],
                        mybir.ActivationFunctionType.Exp,
                        bias=mv,
                        scale=-1.0,
                        accum_out=Z,
                    )
                    rz = vecs.tile([128, 1], F32, tag="rz", name=f"rz_{s}_{m}")
                    nc.vector.reciprocal(rz, Z)
                    sc = vecs.tile([128, 1], F32, tag="sc", name=f"sc_{s}_{m}")
                    nc.vector.tensor_mul(sc, rz, gbc)  # gamma / Z
                    scales.append(sc)
                st[s]["expn"] = expn
                st[s]["scales"] = scales

            def expTf(s):
                # fp16 transposes of the weights; the PSUM->SBUF evacuation
                # casts to fp8 in DoubleRow lhsT layout:
                # et[h][p, o, cb*128+c] = exp[cb-block][c, 128*(2h+o)+p]
                expn = st[s]["expn"]
                expT = []
                for h in range(2):
                    bounce = ps_o.tile([128, 2, CB, 128], F16, tag="acc",
                                       name=f"ebounce_{s}_{h}")
                    for o in range(2):
                        j = 2 * h + o
                        for cb in range(CB):
                            nc.tensor.transpose(
                                bounce[:, o, cb, :],
                                expn[:, cb, j * 128 : (j + 1) * 128],
                                ident,
                            )
                    et = exptp.tile([128, 2, CB * 128], F8, tag="expT",
                                    name=f"expT_{s}_{h}")
                    if h % 2 == 0:
                        nc.scalar.copy(et[:], bounce[:, :, :, :])
                    else:
                        nc.vector.tensor_copy(et[:], bounce[:, :, :, :])
                    expT.append(et)
                st[s]["expT"] = expT

            def aphase(s, cbs=range(CB)):
                # psum = exp @ q via fp8 DoubleRow matmuls; epilogue
                # out_fp16 = psum * (gamma/Z) alternating ACT / DVE;
                # staged 0.5MB fp16 DMAs (host adds the +x residual)
                q8 = st[s]["q8"]
                expT, scales = st[s]["expT"], st[s]["scales"]
                ostage = {}
                for cb in cbs:
                    for no in range(NO):
                        acc = ps_o.tile([128, 512], F32, tag="acc",
                                        name=f"acc_{s}_{no}_{cb}")
                        for h in range(2):
                            nc.tensor.matmul(
                                acc[:],
                                lhsT=expT[h][:, :, cb * 128 : (cb + 1) * 128],
                                rhs=q8[h][:, :, no * 512 : (no + 1) * 512],
                                start=(h == 0),
                                stop=(h == 1),
                                perf_mode=DR,
                            )
                        half = no // (NO // 2)
                        if (cb, half) not in ostage:
                            ot = outsp.tile([128, (NO // 2) * 512], F16,
                                            tag="ot", name=f"ot_{s}_{cb}_{half}")
                            ostage[(cb, half)] = ot
                        ot = ostage[(cb, half)]
                        osl = slice((no % (NO // 2)) * 512,
                                    (no % (NO // 2) + 1) * 512)
                        if no % 2 == 0:
                            nc.scalar.activation(
                                ot[:, osl], acc[:],
                                mybir.ActivationFunctionType.Copy,
                                scale=scales[cb],
                            )
                        else:
                            nc.vector.tensor_scalar_mul(
                                ot[:, osl], acc[:], scales[cb])
                        if no % (NO // 2) == NO // 2 - 1:
                            nc.sync.dma_start(
                                out=out_ap[
                                    s * C + cb * 128 : s * C + (cb + 1) * 128,
                                    half * (NO // 2) * 512 :
                                    (half + 1) * (NO // 2) * 512,
                                ],
                                in_=ot[:],
                            )

            # ---- interleaved emission schedule -----------------------
            # front window only needs xt(0), xt(1), x8(0); x8(1) is
            # deferred so the E phase isn't DMA-saturated
            loadT(0)
            loadT(1)
            load8(0)
            for k in range(NT):
                emm(0, k)
            softmax(0)
            expTf(0)
            load8(1)
            for k in range(NT):
                emm(1, k)
            # A(s0) emitted around softmax(1) so its matmuls fill the PE
            # while sample-1's softmax chain runs on ACT/DVE
            aphase(0, range(0, 2))
            softmax(1)
            aphase(0, range(2, CB))
            expTf(1)
            aphase(1)
    return nc


def _split_excess_waits(nc, max_waits=1):
    """This container's walrus rejects >1 sync-wait on one instruction
    ("Too many sync wait commands"); hoist extras onto standalone
    InstEventSemaphore preludes on the same engine."""
    n = 0
    for fn in nc.m.functions:
        for bb in fn.blocks:
            out = []
            for inst in bb.instructions:
                si = inst.sync_info
                if si is not None and si.on_wait and len(si.on_wait) > max_waits:
                    waits = list(si.on_wait)
                    head, keep = waits[:-max_waits], waits[-max_waits:]
                    for i, w in enumerate(head):
                        ev = mybir.InstEventSemaphore(
                            name=f"{inst.name}-wsplit{i}", ins=[], outs=[])
                        ev.engine = inst.engine
                        ev.sync_info = mybir.SyncInfo(on_wait=[w], on_update=[])
                        out.append(ev)
                        n += 1
                    inst.sync_info = mybir.SyncInfo(
                        on_wait=keep, on_update=list(si.on_update))
                out.append(inst)
            bb.instructions[:] = out
    return n


_cache = {}


def _get_nc():
    if 'nc' not in _cache:
        nc = bass.Bass()
        build(nc)
        _split_excess_waits(nc)
        _cache['nc'] = nc
    return _cache['nc']


def make_in_maps(x: np.ndarray, gamma: np.ndarray, n_cores: int = 8):
    import ml_dtypes

    B, CH, H, W = x.shape          # (16, 512, 64, 64)
    NSP = H * W
    SS = B // n_cores
    g = np.ascontiguousarray(gamma, dtype=np.float32).reshape(1, 1)
    x16 = np.ascontiguousarray(x, dtype=np.float16).reshape(B, CH, NSP)
    # A-phase rhs: e4m3 (|x|<=5.5 so OCP e4m3fn bytes == TRN fp8e4)
    x8 = x16.astype(ml_dtypes.float8_e4m3fn)
    xt16 = np.ascontiguousarray(x16.transpose(0, 2, 1))   # [B, N, C]
    return [
        {
            "xt": np.ascontiguousarray(
                xt16[i * SS : (i + 1) * SS].reshape(SS * NSP // 2, 2 * CH)
            ),
            "x8": np.ascontiguousarray(
                x8[i * SS : (i + 1) * SS].reshape(SS * CH, NSP)
            ),
            "gamma": g,
        }
        for i in range(n_cores)
    ]


def kernel(x: np.ndarray, gamma: np.ndarray) -> np.ndarray:
    from concourse.bass_utils import run_bass_kernel_spmd

    B, CH, H, W = x.shape          # (16, 512, 64, 64)
    M = 8                          # cores
    SS = B // M                    # samples per core
    nc = _get_nc()
    in_maps = make_in_maps(x, gamma, M)
    res = run_bass_kernel_spmd(nc, in_maps, core_ids=list(range(M)))
    out = np.concatenate(
        [
            res.results[i]["out"].astype(np.float32).reshape(SS, CH, H, W)
            for i in range(M)
        ],
        axis=0,
    )
    # residual added host-side in f32
    return np.ascontiguousarray(out + x.astype(np.float32), dtype=np.float32)
